# revision 41
# baseline (speedup 1.0000x reference)
"""Sparse masked attention layer for Trainium2, sharded over 8 NeuronCores.

Strategy
--------
Only token positions present in ``mask_ind`` participate (columns not in the
set get -inf pre-softmax; rows not in the set are zeroed post-softmax), so
the host compacts each batch to its kept token set, the device runs dense
attention on the compacted tokens, and the host scatters results back,
filling non-kept rows with ``bproj``.

Sharding: core = (batch b, head-group g) -> 4 batches x 2 groups of 8 heads.
Each core computes q/k/v projections for its 8 heads, attention per head,
and its partial contribution to the output projection (rows g*512:(g+1)*512
of Wproj).  The two partials of a batch are summed on the host.

Performance structure:
  * all matmuls in bf16 (fp32 PSUM accumulate); the tiny reciprocal
    broadcast runs f32r (full-rate, exact fp32 bits).
  * S for a head pair is row-tiled on the PE (K=64 strips at partitions
    0-63 / 64-127, concurrent) into ONE 2-bank PSUM tile [128, 2, 512], so
    exp for both heads is a single merged ACT call (halves the ~352-cycle
    per-call overhead on the Scalar engine, the kernel's bottleneck).
  * exp output pT is bf16 and feeds the AV matmul directly; the AV "keep"
    column computes the softmax denominator for free.
  * one software pipeline over (group, key-chunk) steps: S is emitted one
    step ahead (across q-group boundaries too, at raised scheduler
    priority) so the Scalar engine never waits; V chunks, the next head
    pair's QK projections and the first output-projection columns ride in
    filler slots inside ACT-bound loops, keeping the HAM clock-gate warm.
  * softmax normalization is deferred to the next group's window: AV
    accumulators drain to SBUF immediately (freeing PSUM banks), then the
    reciprocal / f32r broadcast / multiply chain overlaps the next loop.
  * prefix is minimal: only the two projection groups S(g0, kc 0..3) needs
    run before the pipeline starts; the rest arrive just-in-time as
    fillers.  Output is bf16, staged per column block, summed on host.
"""

from collections import deque

import numpy as np

B, C, D, H = 4, 2048, 1024, 16
HD = D // H          # 64
HPC = H // 2         # 8 heads per core
FQ = HPC * HD        # 512 per-core q/k/v feature count
N_CORES = 8

_NC_CACHE = {}


def _chunks(total, step):
    return [(i, min(step, total - i)) for i in range(0, total, step)]


def _build_nc(Cp, Cq, has_bias):
    import concourse.mybir as mybir
    import concourse.tile as tile
    from concourse import bacc

    f32 = mybir.dt.float32
    f32r = mybir.dt.float32r
    bf16 = mybir.dt.bfloat16
    Exp = mybir.ActivationFunctionType.Exp

    NC = Cp // 128       # key chunks of 128
    KD = D // 128        # 8 contraction chunks for the projections
    nA = _chunks(Cp, 512)      # projection moving groups (full padded width)
    qgroups = _chunks(Cq, 512) # attention q groups (trimmed to real tokens)
    NG = len(qgroups)

    nc = bacc.Bacc()
    xT = nc.dram_tensor("xT", [D, Cp], bf16, kind="ExternalInput")
    wqk = nc.dram_tensor("wqk", [D, 2 * FQ], bf16, kind="ExternalInput")
    wv = nc.dram_tensor("wv", [D, FQ], bf16, kind="ExternalInput")
    wp = nc.dram_tensor("wp", [FQ, D], bf16, kind="ExternalInput")
    keep = nc.dram_tensor("keep", [128, NC], f32, kind="ExternalInput")
    onesf = nc.dram_tensor("onesf", [1, 64], f32r, kind="ExternalInput")
    if has_bias:
        bqkT = nc.dram_tensor("bqkT", [128, 8], f32, kind="ExternalInput")
        bvb = nc.dram_tensor("bvb", [128, FQ], f32, kind="ExternalInput")
    outT = nc.dram_tensor("outT", [D, Cq], bf16, kind="ExternalOutput")

    with tile.TileContext(nc) as tc:
        with (
            tc.tile_pool(name="inp", bufs=1) as p_in,
            tc.tile_pool(name="big", bufs=1) as p_big,
            tc.tile_pool(name="pT", bufs=4) as p_pT,
            tc.tile_pool(name="att", bufs=3) as p_att,
            tc.tile_pool(name="outs", bufs=4) as p_out,
        ):
            qkT = p_big.tile([128, 8, Cp], bf16)
            vsb = p_big.tile([128, NC, HPC, HD + 1], bf16)
            attnT = p_big.tile([128, HPC // 2, Cq], bf16)

            xTs = p_in.tile([128, KD, Cp], bf16)
            wqks = p_in.tile([128, KD, 2 * FQ], bf16)
            wvs = p_in.tile([128, KD, FQ], bf16)
            # prefix-critical slices first: the opening projection groups
            # read x columns [0:512) and the m=0 / m=4 weight blocks only
            wqkr = wqk[:].rearrange("(k p) n -> p k n", p=128)
            nc.sync.dma_start(wqks[:, :, 0:128], wqkr[:, :, 0:128])
            nc.sync.dma_start(wqks[:, :, 512:640], wqkr[:, :, 512:640])
            for k in range(KD):
                nc.sync.dma_start(xTs[:, k, 0:512],
                                  xT[k * 128:(k + 1) * 128, 0:512])
            for k in range(KD):
                nc.sync.dma_start(xTs[:, k, 512:Cp],
                                  xT[k * 128:(k + 1) * 128, 512:Cp])
            nc.sync.dma_start(wqks[:, :, 128:512], wqkr[:, :, 128:512])
            nc.sync.dma_start(wqks[:, :, 640:1024], wqkr[:, :, 640:1024])
            wvr = wv[:].rearrange("(k p) n -> p k n", p=128)
            nc.sync.dma_start(wvs[:, 0:4], wvr[:, 0:4])
            nc.sync.dma_start(wvs[:, 4:8], wvr[:, 4:8])
            keeps = p_in.tile([128, NC], f32)
            nc.sync.dma_start(keeps[:], keep[:])
            onesfs = p_in.tile([1, 64], f32r)
            nc.sync.dma_start(onesfs[:], onesf[:])
            wps = p_in.tile([128, HPC // 2, D], bf16)
            nc.sync.dma_start(wps[:], wp[:].rearrange("(c p) n -> p c n", p=128))
            if has_bias:
                bqkTs = p_in.tile([128, 8], f32)
                nc.sync.dma_start(bqkTs[:], bqkT[:])
                bvbs = p_in.tile([128, FQ], f32)
                nc.sync.dma_start(bvbs[:], bvb[:])

            c_tail = list(range(NG))

            with (
                tc.tile_pool(name="psA", bufs=2, space="PSUM") as psA,
                tc.tile_pool(name="psS", bufs=2, space="PSUM") as psS,
                tc.tile_pool(name="psAV", bufs=2, space="PSUM") as psAV,
            ):
                # qkT[f, c] = (x @ Wqk)^T for one 128-feature chunk m.
                def emit_qk_group(m, n0, nsz):
                    ps = psA.tile([128, 512], f32, tag="psA")
                    for k in range(KD):
                        nc.tensor.matmul(
                            ps[:, :nsz],
                            wqks[:, k, m * 128:(m + 1) * 128],
                            xTs[:, k, n0:n0 + nsz],
                            start=(k == 0), stop=(k == KD - 1),
                        )
                    if has_bias:
                        nc.vector.tensor_scalar_add(
                            qkT[:, m, n0:n0 + nsz], ps[:, :nsz],
                            bqkTs[:, m:m + 1]
                        )
                    else:
                        nc.vector.tensor_copy(
                            qkT[:, m, n0:n0 + nsz], ps[:, :nsz])

                # v[token, 2 heads] = (x @ Wv-slice) * keep for one
                # 128-token chunk c and one head pair vp (128 features)
                def emit_v_pair(c, vp):
                    ps = psA.tile([128, 512], f32, tag="psA")
                    for k in range(KD):
                        nc.tensor.matmul(
                            ps[:, 0:128],
                            xTs[:, k, c * 128:(c + 1) * 128],
                            wvs[:, k, vp * 128:(vp + 1) * 128],
                            start=(k == 0), stop=(k == KD - 1),
                        )
                    dst = vsb[:, c, 2 * vp:2 * vp + 2, 0:HD]
                    if has_bias:
                        tmp = p_att.tile([128, FQ], f32, tag="vtmp")
                        nc.vector.tensor_add(
                            tmp[:, 0:128], ps[:, 0:128],
                            bvbs[:, vp * 128:(vp + 1) * 128])
                        nc.vector.tensor_scalar_mul(
                            dst, tmp[:, 0:128], keeps[:, c:c + 1]
                        )
                    else:
                        nc.vector.tensor_scalar_mul(
                            dst, ps[:, 0:128], keeps[:, c:c + 1]
                        )

                # one output-projection column block: out[m-chunk, n-slice]
                def emit_c_group(m, gi, pool):
                    n0, nsz = qgroups[gi]
                    ps = pool.tile([128, 512], f32, tag="psA")
                    for j in range(HPC // 2):
                        nc.tensor.matmul(
                            ps[:, :nsz],
                            wps[:, j, m * 128:(m + 1) * 128],
                            attnT[:, j, n0:n0 + nsz],
                            start=(j == 0), stop=(j == HPC // 2 - 1),
                        )
                    st = p_out.tile([128, 512], bf16, tag="st")
                    if (m + gi) % 2 == 0:
                        nc.vector.tensor_copy(st[:, :nsz], ps[:, :nsz])
                    else:
                        nc.scalar.copy(st[:, :nsz], ps[:, :nsz])
                    nc.sync.dma_start(
                        outT[m * 128:(m + 1) * 128, n0:n0 + nsz], st[:, :nsz]
                    )

                # keep columns (denominator rides row 64 of the AV output)
                for j in range(HPC):
                    nc.vector.tensor_copy(vsb[:, :, j, HD:HD + 1], keeps[:])

                # prefix: just enough projection for S(group 0, kc 0..3)
                emit_qk_group(4, *nA[0])
                emit_qk_group(0, *nA[0])

                def emit_S(hp, q0, qsz, kc, ss):
                    for hi in range(2):
                        lo = hi * 64
                        nc.tensor.matmul(
                            ss[:, hi, :qsz],
                            qkT[lo:lo + 64, 4 + hp, kc * 128:(kc + 1) * 128],
                            qkT[lo:lo + 64, hp, q0:q0 + qsz],
                            start=True, stop=True,
                        )

                # deferred normalization: out = av[0:64] / av[64].
                # The DVE part runs at the top of the FOLLOWING group (av
                # drains to SBUF immediately, freeing PSUM banks); the
                # recip-gated broadcast matmuls are emitted only after
                # AV(1)/AV(2) so they never block the boundary.
                def norm_dve(prev):
                    avs, hp, q0, qsz = prev
                    st = []
                    for hi in range(2):
                        dcp = p_att.tile([1, 512], f32, tag=f"dcp{hi}")
                        nc.vector.tensor_copy(
                            dcp[0:1, :qsz], avs[hi][64:65, :qsz])
                        avsb = p_att.tile([64, 512], f32, tag=f"avsb{hi}")
                        nc.vector.tensor_copy(
                            avsb[:, :qsz], avs[hi][0:64, :qsz])
                        rec = p_att.tile([1, 512], f32, tag=f"rec{hi}")
                        nc.vector.reciprocal_approx_fast(
                            rec[0:1, :qsz], dcp[0:1, :qsz])
                        recr = p_att.tile([1, 512], f32r, tag=f"recr{hi}")
                        nc.vector.tensor_copy(recr[0:1, :qsz], rec[0:1, :qsz])
                        st.append((avsb, recr))
                    return st

                def norm_head(prev, st, hi):
                    _, hp, q0, qsz = prev
                    avsb, recr = st[hi]
                    bcp = psA.tile([128, 512], f32, tag="psA")
                    nc.tensor.matmul(bcp[0:64, :qsz], onesfs[0:1, :],
                                     recr[0:1, :qsz], start=True, stop=True)
                    bcs = p_att.tile([64, 512], f32, tag=f"bcs{hi}")
                    nc.vector.tensor_copy(bcs[:, :qsz], bcp[0:64, :qsz])
                    lo = hi * 64
                    nc.vector.tensor_mul(
                        attnT[lo:lo + 64, hp, q0:q0 + qsz],
                        avsb[:, :qsz],
                        bcs[:, :qsz],
                    )

                groups = [(hp, q0, qsz)
                          for hp in range(4) for q0, qsz in qgroups]
                NGRP = len(groups)

                # filler work, per group, in deques of (kind, args)
                fillers = [deque() for _ in range(NGRP)]
                fill_rate = [1] * NGRP
                fill_start = [0] * NGRP
                fill_stride = [1] * NGRP
                # group 0: pair-0 V chunks just-in-time interleaved with
                # the deferred projection groups of head pair 0
                g0w = [("v", (0, 0))]
                laterq = [(4, n0, nsz) for n0, nsz in nA[1:]] + \
                         [(0, n0, nsz) for n0, nsz in nA[1:]]
                for c in range(1, NC):
                    if laterq:
                        g0w.append(("qk", laterq.pop(0)))
                    g0w.append(("v", (c, 0)))
                g0w.extend(("qk", a) for a in laterq)
                fillers[0].extend(g0w)
                fill_rate[0] = 2
                # q/k projections and V slices for the next head pair
                # spread over the current pair's groups
                for hp in range(3):
                    work = [("qk", (m, n0, nsz))
                            for m in (hp + 1, 4 + hp + 1) for n0, nsz in nA]
                    work += [("v", (c, hp + 1)) for c in range(NC)]
                    tgt = [hp * NG + gg for gg in range(NG)]
                    if hp == 0:
                        tgt = tgt[1:] or tgt
                    for i, w in enumerate(work):
                        gf = tgt[i % len(tgt)]
                        fillers[gf].append(w)
                        fill_start[gf] = 1
                        fill_stride[gf] = 1
                # output projection for q-range j rides in group
                # NGRP-NG+1+j, right after that range's pair-3 norm
                if NG >= 2:
                    for j in range(NG - 1):
                        gf = NGRP - NG + 1 + j
                        for m in range(8):
                            fillers[gf].append(("c", (m, j)))
                        fill_start[gf] = 3
                        fill_stride[gf] = 1
                    c_tail = [NG - 1]

                def pop_filler(g, kc=None):
                    kind, args = fillers[g].popleft()
                    if kind == "v":
                        emit_v_pair(*args)
                    elif kind == "qk":
                        emit_qk_group(*args)
                    else:
                        emit_c_group(*args, pool=psA)

                # ---------------- the main pipeline ----------------
                hp0, q00, qsz0 = groups[0]
                pending_S = psS.tile([128, 2, 512], f32, tag="ss")
                emit_S(hp0, q00, qsz0, 0, pending_S)

                prev = None
                prev_st = None
                ndone = 0
                for g, (hp, q0, qsz) in enumerate(groups):
                    if prev is not None:
                        prev_st = norm_dve(prev)
                        ndone = 0
                    avs = [
                        psAV.tile([65, 512], f32, tag="av",
                                  name=f"av_{g}_{hi}")
                        for hi in range(2)
                    ]
                    for kc in range(NC):
                        ss_cur = pending_S
                        pT = p_pT.tile([128, 2, 512], bf16, tag="pT")
                        nc.scalar.activation(
                            pT[:, :, :qsz], ss_cur[:, :, :qsz], Exp,
                            scale=0.125
                        )
                        # emit the next S (possibly the next group's first)
                        if kc + 1 < NC:
                            nxt = (g, kc + 1)
                        elif g + 1 < NGRP:
                            nxt = (g + 1, 0)
                        else:
                            nxt = None
                        if nxt is not None:
                            g2, kc2 = nxt
                            if g2 != g:
                                # a new group's S may need queued projection
                                # filler output — drain it first
                                for gd in range(g + 1):
                                    while (fillers[gd]
                                           and fillers[gd][0][0] == "qk"):
                                        pop_filler(gd)
                            hp2, q02, qsz2 = groups[g2]
                            pending_S = psS.tile([128, 2, 512], f32,
                                                 tag="ss")
                            with tc.high_priority(offset=64):
                                emit_S(hp2, q02, qsz2, kc2, pending_S)
                        if (kc >= fill_start[g]
                                and (kc - fill_start[g]) % fill_stride[g]
                                == 0):
                            for _ in range(fill_rate[g]):
                                if fillers[g]:
                                    pop_filler(g, kc)
                        for hi in range(2):
                            nc.tensor.matmul(
                                avs[hi][:, :qsz],
                                vsb[:, kc, 2 * hp + hi, :],
                                pT[:, hi, :qsz],
                                start=(kc == 0), stop=(kc == NC - 1),
                            )
                        if prev is not None and kc in (1, 2):
                            norm_head(prev, prev_st, kc - 1)
                            ndone = kc
                    if prev is not None and ndone < 2:
                        for hi in range(ndone, 2):
                            norm_head(prev, prev_st, hi)
                    while fillers[g]:
                        pop_filler(g)
                    prev = (avs, hp, q0, qsz)

                # tail: last group's normalization
                last_st = norm_dve(prev)
                norm_head(prev, last_st, 0)
                norm_head(prev, last_st, 1)

            # remaining output columns in a fresh quad-buffered PSUM pool
            with tc.tile_pool(name="psC", bufs=4, space="PSUM") as psC:
                for gi in c_tail:
                    for m in range(8):
                        emit_c_group(m, gi, pool=psC)

    nc.finalize()
    return nc


def _get_nc(Cp, Cq, has_bias):
    key = (Cp, Cq, has_bias)
    if key not in _NC_CACHE:
        _NC_CACHE[key] = _build_nc(Cp, Cq, has_bias)
    return _NC_CACHE[key]


def kernel(x, mask_ind, Wqkv, bqkv, Wproj, bproj, **_unused):
    import ml_dtypes
    from concourse.bass_utils import run_bass_kernel_spmd

    bf = ml_dtypes.bfloat16
    x = np.asarray(x, dtype=np.float32)
    mask_ind = np.asarray(mask_ind)
    Wqkv = np.asarray(Wqkv, dtype=np.float32)
    bqkv = np.asarray(bqkv, dtype=np.float32)
    Wproj = np.asarray(Wproj, dtype=np.float32)
    bproj = np.asarray(bproj, dtype=np.float32)

    # kept-token sets per batch (matches reference _keep_mask semantics)
    idx = []
    for b in range(B):
        mi = mask_ind[b]
        mi = mi[mi >= 0]
        mi = np.clip(mi, 0, C - 1)
        idx.append(np.unique(mi).astype(np.int64))
    Cq = max(128, max(len(u) for u in idx))
    Cp = ((Cq + 127) // 128) * 128
    NC = Cp // 128
    has_bias = bool(np.any(bqkv))

    nc = _get_nc(Cp, Cq, has_bias)

    in_maps = []
    for core in range(N_CORES):
        b, g = core // 2, core % 2
        u = idx[b]
        n = len(u)
        xk = np.zeros((Cp, D), dtype=np.float32)
        xk[:n] = x[b, u]
        keep = np.zeros(Cp, dtype=np.float32)
        keep[:n] = 1.0
        qs, ks, vs = g * FQ, D + g * FQ, 2 * D + g * FQ
        wqk = np.concatenate(
            [Wqkv[:, qs:qs + FQ], Wqkv[:, ks:ks + FQ]], axis=1
        )
        im = {
            "xT": np.ascontiguousarray(xk.T).astype(bf),
            "wqk": np.ascontiguousarray(wqk).astype(bf),
            "wv": np.ascontiguousarray(Wqkv[:, vs:vs + FQ]).astype(bf),
            "wp": np.ascontiguousarray(
                Wproj[g * FQ:(g + 1) * FQ, :]).astype(bf),
            "keep": np.ascontiguousarray(keep.reshape(NC, 128).T),
            "onesf": np.ones((1, 64), dtype=np.float32),
        }
        if has_bias:
            bqk = np.concatenate([bqkv[qs:qs + FQ], bqkv[ks:ks + FQ]])
            im["bqkT"] = np.ascontiguousarray(bqk.reshape(8, 128).T)
            im["bvb"] = np.broadcast_to(
                bqkv[vs:vs + FQ], (128, FQ)).astype(np.float32).copy()
        in_maps.append(im)

    global _last_in_maps
    _last_in_maps = in_maps
    res = run_bass_kernel_spmd(nc, in_maps, core_ids=list(range(N_CORES)))

    out = np.broadcast_to(bproj, (B, C, D)).copy()
    for b in range(B):
        u = idx[b]
        n = len(u)
        comb = (res.results[2 * b]["outT"].astype(np.float32)
                + res.results[2 * b + 1]["outT"].astype(np.float32))
        out[b, u] += comb.T[:n]
    return out


# revision 42
# speedup vs baseline: 1.0272x; 1.0272x over previous
"""Sparse masked attention layer for Trainium2, sharded over 8 NeuronCores.

Strategy
--------
Only token positions present in ``mask_ind`` participate (columns not in the
set get -inf pre-softmax; rows not in the set are zeroed post-softmax), so
the host compacts each batch to its kept token set, the device runs dense
attention on the compacted tokens, and the host scatters results back,
filling non-kept rows with ``bproj``.

Sharding: core = (batch b, head-group g) -> 4 batches x 2 groups of 8 heads.
Each core computes q/k/v projections for its 8 heads, attention per head,
and its partial contribution to the output projection (rows g*512:(g+1)*512
of Wproj).  The two partials of a batch are summed on the host.

Performance structure:
  * all matmuls in bf16 (fp32 PSUM accumulate); the tiny reciprocal
    broadcast runs f32r (full-rate, exact fp32 bits).
  * S for a head pair is row-tiled on the PE (K=64 strips at partitions
    0-63 / 64-127, concurrent) into ONE 2-bank PSUM tile [128, 2, 512], so
    exp for both heads is a single merged ACT call (halves the ~352-cycle
    per-call overhead on the Scalar engine, the kernel's bottleneck).
  * exp output pT is bf16 and feeds the AV matmul directly; the AV "keep"
    column computes the softmax denominator for free.
  * one software pipeline over (group, key-chunk) steps: S is emitted one
    step ahead (across q-group boundaries too, at raised scheduler
    priority) so the Scalar engine never waits; V chunks, the next head
    pair's QK projections and the first output-projection columns ride in
    filler slots inside ACT-bound loops, keeping the HAM clock-gate warm.
  * softmax normalization is deferred to the next group's window: AV
    accumulators drain to SBUF immediately (freeing PSUM banks), then the
    reciprocal / f32r broadcast / multiply chain overlaps the next loop.
  * prefix is minimal: only the two projection groups S(g0, kc 0..3) needs
    run before the pipeline starts; the rest arrive just-in-time as
    fillers.  Output is bf16, staged per column block, summed on host.
"""

from collections import deque

import numpy as np

B, C, D, H = 4, 2048, 1024, 16
HD = D // H          # 64
HPC = H // 2         # 8 heads per core
FQ = HPC * HD        # 512 per-core q/k/v feature count
N_CORES = 8

_NC_CACHE = {}


def _chunks(total, step):
    return [(i, min(step, total - i)) for i in range(0, total, step)]


def _build_nc(Cp, Cq, has_bias):
    import concourse.mybir as mybir
    import concourse.tile as tile
    from concourse import bacc

    f32 = mybir.dt.float32
    f32r = mybir.dt.float32r
    bf16 = mybir.dt.bfloat16
    Exp = mybir.ActivationFunctionType.Exp

    NC = Cp // 128       # key chunks of 128
    KD = D // 128        # 8 contraction chunks for the projections
    nA = _chunks(Cp, 512)      # projection moving groups (full padded width)
    qgroups = _chunks(Cq, 512) # attention q groups (trimmed to real tokens)
    NG = len(qgroups)

    nc = bacc.Bacc()
    xT = nc.dram_tensor("xT", [D, Cp], bf16, kind="ExternalInput")
    wqk = nc.dram_tensor("wqk", [D, 2 * FQ], bf16, kind="ExternalInput")
    wv = nc.dram_tensor("wv", [D, FQ], bf16, kind="ExternalInput")
    wp = nc.dram_tensor("wp", [FQ, D], bf16, kind="ExternalInput")
    keep = nc.dram_tensor("keep", [128, NC], f32, kind="ExternalInput")
    onesf = nc.dram_tensor("onesf", [1, 64], f32r, kind="ExternalInput")
    if has_bias:
        bqkT = nc.dram_tensor("bqkT", [128, 8], f32, kind="ExternalInput")
        bvb = nc.dram_tensor("bvb", [128, FQ], f32, kind="ExternalInput")
    outT = nc.dram_tensor("outT", [D, Cq], bf16, kind="ExternalOutput")

    with tile.TileContext(nc) as tc:
        with (
            tc.tile_pool(name="inp", bufs=1) as p_in,
            tc.tile_pool(name="big", bufs=1) as p_big,
            tc.tile_pool(name="pT", bufs=4) as p_pT,
            tc.tile_pool(name="att", bufs=3) as p_att,
            tc.tile_pool(name="outs", bufs=4) as p_out,
        ):
            qkT = p_big.tile([128, 8, Cp], bf16)
            vsb = p_big.tile([128, NC, HPC, HD + 1], bf16)
            attnT = p_big.tile([128, HPC // 2, Cq], bf16)

            xTs = p_in.tile([128, KD, Cp], bf16)
            wqks = p_in.tile([128, KD, 2 * FQ], bf16)
            wvs = p_in.tile([128, KD, FQ], bf16)
            for k in range(KD):
                nc.sync.dma_start(xTs[:, k], xT[k * 128:(k + 1) * 128, :])
                nc.sync.dma_start(wqks[:, k], wqk[k * 128:(k + 1) * 128, :])
            wvr = wv[:].rearrange("(k p) n -> p k n", p=128)
            nc.sync.dma_start(wvs[:, 0:4], wvr[:, 0:4])
            nc.sync.dma_start(wvs[:, 4:8], wvr[:, 4:8])
            keeps = p_in.tile([128, NC], f32)
            nc.sync.dma_start(keeps[:], keep[:])
            onesfs = p_in.tile([1, 64], f32r)
            nc.sync.dma_start(onesfs[:], onesf[:])
            wps = p_in.tile([128, HPC // 2, D], bf16)
            nc.sync.dma_start(wps[:], wp[:].rearrange("(c p) n -> p c n", p=128))
            if has_bias:
                bqkTs = p_in.tile([128, 8], f32)
                nc.sync.dma_start(bqkTs[:], bqkT[:])
                bvbs = p_in.tile([128, FQ], f32)
                nc.sync.dma_start(bvbs[:], bvb[:])

            c_tail = list(range(NG))

            with (
                tc.tile_pool(name="psA", bufs=2, space="PSUM") as psA,
                tc.tile_pool(name="psS", bufs=2, space="PSUM") as psS,
                tc.tile_pool(name="psAV", bufs=2, space="PSUM") as psAV,
            ):
                # qkT[f, c] = (x @ Wqk)^T for one 128-feature chunk m.
                def emit_qk_group(m, n0, nsz):
                    ps = psA.tile([128, 512], f32, tag="psA")
                    for k in range(KD):
                        nc.tensor.matmul(
                            ps[:, :nsz],
                            wqks[:, k, m * 128:(m + 1) * 128],
                            xTs[:, k, n0:n0 + nsz],
                            start=(k == 0), stop=(k == KD - 1),
                        )
                    if has_bias:
                        nc.vector.tensor_scalar_add(
                            qkT[:, m, n0:n0 + nsz], ps[:, :nsz],
                            bqkTs[:, m:m + 1]
                        )
                    else:
                        nc.vector.tensor_copy(
                            qkT[:, m, n0:n0 + nsz], ps[:, :nsz])

                # v[token, 2 heads] = (x @ Wv-slice) * keep for one
                # 128-token chunk c and one head pair vp (128 features)
                def emit_v_pair(c, vp):
                    ps = psA.tile([128, 512], f32, tag="psA")
                    for k in range(KD):
                        nc.tensor.matmul(
                            ps[:, 0:128],
                            xTs[:, k, c * 128:(c + 1) * 128],
                            wvs[:, k, vp * 128:(vp + 1) * 128],
                            start=(k == 0), stop=(k == KD - 1),
                        )
                    dst = vsb[:, c, 2 * vp:2 * vp + 2, 0:HD]
                    if has_bias:
                        tmp = p_att.tile([128, FQ], f32, tag="vtmp")
                        nc.vector.tensor_add(
                            tmp[:, 0:128], ps[:, 0:128],
                            bvbs[:, vp * 128:(vp + 1) * 128])
                        nc.vector.tensor_scalar_mul(
                            dst, tmp[:, 0:128], keeps[:, c:c + 1]
                        )
                    else:
                        nc.vector.tensor_scalar_mul(
                            dst, ps[:, 0:128], keeps[:, c:c + 1]
                        )

                # one output-projection column block: out[m-chunk, n-slice]
                def emit_c_group(m, gi, pool):
                    n0, nsz = qgroups[gi]
                    ps = pool.tile([128, 512], f32, tag="psA")
                    for j in range(HPC // 2):
                        nc.tensor.matmul(
                            ps[:, :nsz],
                            wps[:, j, m * 128:(m + 1) * 128],
                            attnT[:, j, n0:n0 + nsz],
                            start=(j == 0), stop=(j == HPC // 2 - 1),
                        )
                    st = p_out.tile([128, 512], bf16, tag="st")
                    if (m + gi) % 2 == 0:
                        nc.vector.tensor_copy(st[:, :nsz], ps[:, :nsz])
                    else:
                        nc.scalar.copy(st[:, :nsz], ps[:, :nsz])
                    nc.sync.dma_start(
                        outT[m * 128:(m + 1) * 128, n0:n0 + nsz], st[:, :nsz]
                    )

                # keep columns (denominator rides row 64 of the AV output)
                for j in range(HPC):
                    nc.vector.tensor_copy(vsb[:, :, j, HD:HD + 1], keeps[:])

                # prefix: just enough projection for S(group 0, kc 0..3)
                emit_qk_group(4, *nA[0])
                emit_qk_group(0, *nA[0])

                def emit_S(hp, q0, qsz, kc, ss):
                    for hi in range(2):
                        lo = hi * 64
                        nc.tensor.matmul(
                            ss[:, hi, :qsz],
                            qkT[lo:lo + 64, 4 + hp, kc * 128:(kc + 1) * 128],
                            qkT[lo:lo + 64, hp, q0:q0 + qsz],
                            start=True, stop=True,
                        )

                # deferred normalization: out = av[0:64] / av[64].
                # The DVE part runs at the top of the FOLLOWING group (av
                # drains to SBUF immediately, freeing PSUM banks); the
                # recip-gated broadcast matmuls are emitted only after
                # AV(1)/AV(2) so they never block the boundary.
                def norm_dve(prev):
                    avs, hp, q0, qsz = prev
                    st = []
                    for hi in range(2):
                        dcp = p_att.tile([1, 512], f32, tag=f"dcp{hi}")
                        nc.vector.tensor_copy(
                            dcp[0:1, :qsz], avs[hi][64:65, :qsz])
                        avsb = p_att.tile([64, 512], f32, tag=f"avsb{hi}")
                        nc.vector.tensor_copy(
                            avsb[:, :qsz], avs[hi][0:64, :qsz])
                        rec = p_att.tile([1, 512], f32, tag=f"rec{hi}")
                        nc.vector.reciprocal_approx_fast(
                            rec[0:1, :qsz], dcp[0:1, :qsz])
                        recr = p_att.tile([1, 512], f32r, tag=f"recr{hi}")
                        nc.vector.tensor_copy(recr[0:1, :qsz], rec[0:1, :qsz])
                        st.append((avsb, recr))
                    return st

                def norm_head(prev, st, hi):
                    _, hp, q0, qsz = prev
                    avsb, recr = st[hi]
                    bcp = psA.tile([128, 512], f32, tag="psA")
                    nc.tensor.matmul(bcp[0:64, :qsz], onesfs[0:1, :],
                                     recr[0:1, :qsz], start=True, stop=True)
                    bcs = p_att.tile([64, 512], f32, tag=f"bcs{hi}")
                    nc.vector.tensor_copy(bcs[:, :qsz], bcp[0:64, :qsz])
                    lo = hi * 64
                    nc.vector.tensor_mul(
                        attnT[lo:lo + 64, hp, q0:q0 + qsz],
                        avsb[:, :qsz],
                        bcs[:, :qsz],
                    )

                groups = [(hp, q0, qsz)
                          for hp in range(4) for q0, qsz in qgroups]
                NGRP = len(groups)

                # filler work, per group, in deques of (kind, args)
                fillers = [deque() for _ in range(NGRP)]
                fill_rate = [1] * NGRP
                fill_start = [0] * NGRP
                fill_stride = [1] * NGRP
                # group 0: pair-0 V chunks just-in-time interleaved with
                # the deferred projection groups of head pair 0
                g0w = [("v", (0, 0))]
                laterq = [(4, n0, nsz) for n0, nsz in nA[1:]] + \
                         [(0, n0, nsz) for n0, nsz in nA[1:]]
                for c in range(1, NC):
                    if laterq:
                        g0w.append(("qk", laterq.pop(0)))
                    g0w.append(("v", (c, 0)))
                g0w.extend(("qk", a) for a in laterq)
                fillers[0].extend(g0w)
                fill_rate[0] = 2
                # q/k projections and V slices for the next head pair
                # spread over the current pair's groups
                for hp in range(3):
                    work = [("qk", (m, n0, nsz))
                            for m in (hp + 1, 4 + hp + 1) for n0, nsz in nA]
                    work += [("v", (c, hp + 1)) for c in range(NC)]
                    tgt = [hp * NG + gg for gg in range(NG)]
                    if hp == 0:
                        tgt = tgt[1:] or tgt
                    for i, w in enumerate(work):
                        gf = tgt[i % len(tgt)]
                        fillers[gf].append(w)
                        fill_start[gf] = 1
                        fill_stride[gf] = 1
                # output projection for q-range j rides in group
                # NGRP-NG+1+j, right after that range's pair-3 norm
                if NG >= 2:
                    for j in range(NG - 1):
                        gf = NGRP - NG + 1 + j
                        for m in range(8):
                            fillers[gf].append(("c", (m, j)))
                        fill_start[gf] = 3
                        fill_stride[gf] = 1
                    c_tail = [NG - 1]

                def pop_filler(g, kc=None):
                    kind, args = fillers[g].popleft()
                    if kind == "v":
                        emit_v_pair(*args)
                    elif kind == "qk":
                        emit_qk_group(*args)
                    else:
                        emit_c_group(*args, pool=psA)

                # ---------------- the main pipeline ----------------
                hp0, q00, qsz0 = groups[0]
                pending_S = psS.tile([128, 2, 512], f32, tag="ss")
                emit_S(hp0, q00, qsz0, 0, pending_S)

                prev = None
                prev_st = None
                ndone = 0
                for g, (hp, q0, qsz) in enumerate(groups):
                    if prev is not None:
                        prev_st = norm_dve(prev)
                        ndone = 0
                    avs = [
                        psAV.tile([65, 512], f32, tag="av",
                                  name=f"av_{g}_{hi}")
                        for hi in range(2)
                    ]
                    for kc in range(NC):
                        ss_cur = pending_S
                        pT = p_pT.tile([128, 2, 512], bf16, tag="pT")
                        nc.scalar.activation(
                            pT[:, :, :qsz], ss_cur[:, :, :qsz], Exp,
                            scale=0.125
                        )
                        # emit the next S (possibly the next group's first)
                        if kc + 1 < NC:
                            nxt = (g, kc + 1)
                        elif g + 1 < NGRP:
                            nxt = (g + 1, 0)
                        else:
                            nxt = None
                        if nxt is not None:
                            g2, kc2 = nxt
                            if g2 != g:
                                # a new group's S may need queued projection
                                # filler output — drain it first
                                for gd in range(g + 1):
                                    while (fillers[gd]
                                           and fillers[gd][0][0] == "qk"):
                                        pop_filler(gd)
                            hp2, q02, qsz2 = groups[g2]
                            pending_S = psS.tile([128, 2, 512], f32,
                                                 tag="ss")
                            with tc.high_priority(offset=64):
                                emit_S(hp2, q02, qsz2, kc2, pending_S)
                        if (kc >= fill_start[g]
                                and (kc - fill_start[g]) % fill_stride[g]
                                == 0):
                            for _ in range(fill_rate[g]):
                                if fillers[g]:
                                    pop_filler(g, kc)
                        for hi in range(2):
                            nc.tensor.matmul(
                                avs[hi][:, :qsz],
                                vsb[:, kc, 2 * hp + hi, :],
                                pT[:, hi, :qsz],
                                start=(kc == 0), stop=(kc == NC - 1),
                            )
                        if prev is not None and kc in (1, 2):
                            norm_head(prev, prev_st, kc - 1)
                            ndone = kc
                    if prev is not None and ndone < 2:
                        for hi in range(ndone, 2):
                            norm_head(prev, prev_st, hi)
                    while fillers[g]:
                        pop_filler(g)
                    prev = (avs, hp, q0, qsz)

                # tail: last group's normalization
                last_st = norm_dve(prev)
                norm_head(prev, last_st, 0)
                norm_head(prev, last_st, 1)

            # remaining output columns in a fresh quad-buffered PSUM pool
            with tc.tile_pool(name="psC", bufs=4, space="PSUM") as psC:
                for gi in c_tail:
                    for m in range(8):
                        emit_c_group(m, gi, pool=psC)

    nc.finalize()
    return nc


def _get_nc(Cp, Cq, has_bias):
    key = (Cp, Cq, has_bias)
    if key not in _NC_CACHE:
        _NC_CACHE[key] = _build_nc(Cp, Cq, has_bias)
    return _NC_CACHE[key]


def kernel(x, mask_ind, Wqkv, bqkv, Wproj, bproj, **_unused):
    import ml_dtypes
    from concourse.bass_utils import run_bass_kernel_spmd

    bf = ml_dtypes.bfloat16
    x = np.asarray(x, dtype=np.float32)
    mask_ind = np.asarray(mask_ind)
    Wqkv = np.asarray(Wqkv, dtype=np.float32)
    bqkv = np.asarray(bqkv, dtype=np.float32)
    Wproj = np.asarray(Wproj, dtype=np.float32)
    bproj = np.asarray(bproj, dtype=np.float32)

    # kept-token sets per batch (matches reference _keep_mask semantics)
    idx = []
    for b in range(B):
        mi = mask_ind[b]
        mi = mi[mi >= 0]
        mi = np.clip(mi, 0, C - 1)
        idx.append(np.unique(mi).astype(np.int64))
    Cq = max(128, max(len(u) for u in idx))
    Cp = ((Cq + 127) // 128) * 128
    NC = Cp // 128
    has_bias = bool(np.any(bqkv))

    nc = _get_nc(Cp, Cq, has_bias)

    in_maps = []
    for core in range(N_CORES):
        b, g = core // 2, core % 2
        u = idx[b]
        n = len(u)
        xk = np.zeros((Cp, D), dtype=np.float32)
        xk[:n] = x[b, u]
        keep = np.zeros(Cp, dtype=np.float32)
        keep[:n] = 1.0
        qs, ks, vs = g * FQ, D + g * FQ, 2 * D + g * FQ
        wqk = np.concatenate(
            [Wqkv[:, qs:qs + FQ], Wqkv[:, ks:ks + FQ]], axis=1
        )
        im = {
            "xT": np.ascontiguousarray(xk.T).astype(bf),
            "wqk": np.ascontiguousarray(wqk).astype(bf),
            "wv": np.ascontiguousarray(Wqkv[:, vs:vs + FQ]).astype(bf),
            "wp": np.ascontiguousarray(
                Wproj[g * FQ:(g + 1) * FQ, :]).astype(bf),
            "keep": np.ascontiguousarray(keep.reshape(NC, 128).T),
            "onesf": np.ones((1, 64), dtype=np.float32),
        }
        if has_bias:
            bqk = np.concatenate([bqkv[qs:qs + FQ], bqkv[ks:ks + FQ]])
            im["bqkT"] = np.ascontiguousarray(bqk.reshape(8, 128).T)
            im["bvb"] = np.broadcast_to(
                bqkv[vs:vs + FQ], (128, FQ)).astype(np.float32).copy()
        in_maps.append(im)

    global _last_in_maps
    _last_in_maps = in_maps
    res = run_bass_kernel_spmd(nc, in_maps, core_ids=list(range(N_CORES)))

    out = np.broadcast_to(bproj, (B, C, D)).copy()
    for b in range(B):
        u = idx[b]
        n = len(u)
        comb = (res.results[2 * b]["outT"].astype(np.float32)
                + res.results[2 * b + 1]["outT"].astype(np.float32))
        out[b, u] += comb.T[:n]
    return out
